# revision 7
# baseline (speedup 1.0000x reference)
"""Trainium2 Bass kernel for nn_CCMetrics (connected-component soft-Dice).

Math
----
Reference per sample: probs = softmax(y_pred, ch axis 1) with C=2 channels,
one-hot labels y in {0,1}.  Per-voxel channel sums collapse:
  psum_v = tsum_v = 1          (softmax / one-hot sum to 1 over channels)
  inter_v = probs[true_ch] = sigmoid((2y-1) * (z1 - z0))
So per segment id k (voronoi component, 0..64):
  inter_k = sum of sigmoid values over voxels with id k
  cnt_k   = voxel count with id k
  dice_k  = (2*inter_k + eps) / (2*cnt_k + eps)
  score   = mean over present k in 1..64;  output = mean over batch.

Device algorithm (per core, data-parallel over 4M voxels / 8 cores)
------------------------------------------------------------------
Build two packed streams per voxel (id g, value v = sigmoid(...)):
  z  = g + 0.5                  (exact half-integers)
  x' = g + 0.5 + v              (value stream, thresholds at k+0.5)
Cumulative families, one instruction per bin k (per-partition accumulate):
  R_k = sum relu(x' - (k+0.5))           [ACT Relu + bias + accum]
  T_k = #{x' >= k+0.5} = #{g >= k}       [DVE tensor_scalar is_ge + accum]
  F_k = sum sigmoid(30*(z-(k+0.5)))      [ACT Sigmoid + bias + accum]
        = 0.5*cnt_k + T_{k+1}   (exact to ~1e-13: args are multiples of 30)
Recovery (host, float64):  M_k = R_k - R_{k+1} = inter_k + T_{k+1};
walking k = 64..1 with T_65 = 0: exact T anchors from DVE bins, F bins give
cnt_k = 2*(F_k - T_{k+1}).  ACT pipelines accumulate passes at ~1.3 us while
DVE accumulate passes have a ~4.3 us drain period, so ACT takes the relu
family plus most count bins (sigmoid) and DVE takes preprocessing plus a
spread subset of exact-count anchor bins.
"""

import os
import sys

import numpy as np

for _p in ("/opt/trn_rl_repo",):
    if os.path.isdir(_p) and _p not in sys.path:
        sys.path.insert(0, _p)

from concourse import bacc, bass, mybir, tile  # noqa: E402
from concourse import bass_utils  # noqa: E402

NUM_COMP = 64
EPS = 1e-5
B, C, H, W, D = 2, 2, 128, 128, 128
N = H * W * D
NCORES = 8
CORES_PER_SAMPLE = NCORES // B
CHUNK = N // CORES_PER_SAMPLE
P = 128
F = CHUNK // P
KMAX = NUM_COMP

# Exact-count anchor bins computed on DVE (tensor_scalar is_ge + accum).
# Spread so that sigmoid-chain reconstruction segments stay short.
_nd = int(os.environ.get("CC_ND", "27"))
if _nd >= KMAX:
    DVE_BINS = frozenset(range(1, KMAX + 1))
else:
    # evenly spread anchors from k=KMAX downward
    _step = max(1, round(KMAX / max(_nd, 1)))
    DVE_BINS = frozenset(
        k for k in range(KMAX, 0, -_step)
    ) | {KMAX}
    DVE_BINS = frozenset(sorted(DVE_BINS, reverse=True)[:max(_nd, 1)])
TRACE = False

_prog_cache = {}


def _build_program():
    nc = bacc.Bacc(
        "TRN2",
        target_bir_lowering=False,
        debug=False,
        enable_asserts=False,
        num_devices=NCORES,
    )
    f32 = mybir.dt.float32
    u8 = mybir.dt.uint8

    z0_d = nc.dram_tensor("z0", [P, F], f32, kind="ExternalInput").ap()
    z1_d = nc.dram_tensor("z1", [P, F], f32, kind="ExternalInput").ap()
    y_d = nc.dram_tensor("yb", [P, F], u8, kind="ExternalInput").ap()
    g_d = nc.dram_tensor("vor", [P, F], u8, kind="ExternalInput").ap()
    # bias constants: col j (j=0..63): -(j+1.5) for relu; col 64: 0.0
    negk_d = nc.dram_tensor("negk", [P, KMAX + 1], f32, kind="ExternalInput").ap()
    # sigmoid bias constants: col j: -30*(j+1.5)
    sigb_d = nc.dram_tensor("sigb", [P, KMAX], f32, kind="ExternalInput").ap()
    out_d = nc.dram_tensor("out", [P, 3 * KMAX], f32, kind="ExternalOutput").ap()

    Alu = mybir.AluOpType
    Act = mybir.ActivationFunctionType

    with tile.TileContext(nc) as tc:
        with tc.tile_pool(name="main", bufs=1) as pool:
            z0 = pool.tile([P, F], f32)
            z1 = pool.tile([P, F], f32)
            yt = pool.tile([P, F], u8)
            gt = pool.tile([P, F], u8)
            negk = pool.tile([P, KMAX + 1], f32)
            sigb = pool.tile([P, KMAX], f32)
            nc.sync.dma_start(out=z0[:], in_=z0_d[:])
            nc.sync.dma_start(out=z1[:], in_=z1_d[:])
            nc.sync.dma_start(out=yt[:], in_=y_d[:])
            nc.sync.dma_start(out=gt[:], in_=g_d[:])
            nc.sync.dma_start(out=negk[:], in_=negk_d[:])
            nc.sync.dma_start(out=sigb[:], in_=sigb_d[:])

            # ---- preprocessing (DVE + one ACT sigmoid) ----
            s = pool.tile([P, F], f32)
            nc.vector.tensor_sub(s[:], z1[:], z0[:])
            yf = pool.tile([P, F], f32)
            nc.vector.tensor_scalar(
                out=yf[:], in0=yt[:], scalar1=2.0, scalar2=-1.0,
                op0=Alu.mult, op1=Alu.add,
            )
            t = pool.tile([P, F], f32)
            nc.vector.tensor_mul(t[:], s[:], yf[:])
            v = pool.tile([P, F], f32)
            nc.scalar.activation(
                out=v[:], in_=t[:], func=Act.Sigmoid,
                bias=negk[:, KMAX:KMAX + 1], scale=1.0,  # bias 0.0
            )
            zt = pool.tile([P, F], f32)
            nc.vector.tensor_scalar(
                out=zt[:], in0=gt[:], scalar1=0.5, scalar2=None, op0=Alu.add,
            )
            x = pool.tile([P, F], f32)
            nc.vector.tensor_add(x[:], v[:], zt[:])

            racc = pool.tile([P, KMAX], f32)
            tacc = pool.tile([P, KMAX], f32)
            facc = pool.tile([P, KMAX], f32)
            trash_a = pool.tile([P, F], f32)
            trash_d = pool.tile([P, F], f32)

            for k in range(1, KMAX + 1):
                j = k - 1
                # value family on ACT:  R_k = sum relu(x - (k+0.5))
                nc.scalar.activation(
                    out=trash_a[:], in_=x[:], func=Act.Relu,
                    bias=negk[:, j:j + 1], scale=1.0,
                    accum_out=racc[:, j:j + 1],
                )
                if k in DVE_BINS:
                    # exact count anchor on DVE: T_k = #{x >= k+0.5}
                    nc.vector.tensor_scalar(
                        out=trash_d[:], in0=x[:], scalar1=float(k) + 0.5,
                        scalar2=None, op0=Alu.is_ge, op1=Alu.add,
                        accum_out=tacc[:, j:j + 1],
                    )
                else:
                    # sigmoid count family on ACT over the half-integer stream
                    nc.scalar.activation(
                        out=trash_a[:], in_=zt[:], func=Act.Sigmoid,
                        bias=sigb[:, j:j + 1], scale=30.0,
                        accum_out=facc[:, j:j + 1],
                    )

            nc.sync.dma_start(out=out_d[:, 0:KMAX], in_=racc[:])
            nc.sync.dma_start(out=out_d[:, KMAX:2 * KMAX], in_=tacc[:])
            nc.sync.dma_start(out=out_d[:, 2 * KMAX:3 * KMAX], in_=facc[:])

    nc.compile()
    return nc


def _get_program():
    key = ("prog", tuple(sorted(DVE_BINS)))
    if key not in _prog_cache:
        _prog_cache[key] = _build_program()
    return _prog_cache[key]


def _consts():
    negk = np.concatenate(
        [-(np.arange(1, KMAX + 1, dtype=np.float32) + 0.5), np.zeros(1, np.float32)])
    sigb = -30.0 * (np.arange(1, KMAX + 1, dtype=np.float32) + 0.5)
    return (np.broadcast_to(negk, (P, KMAX + 1)).copy(),
            np.broadcast_to(sigb, (P, KMAX)).copy())


def kernel(y_pred: np.ndarray, y: np.ndarray, voronoi: np.ndarray) -> np.ndarray:
    y_pred = np.asarray(y_pred, dtype=np.float32)
    y = np.asarray(y)
    voronoi = np.asarray(voronoi)

    nc = _get_program()
    negk, sigb = _consts()

    in_maps = []
    for c in range(NCORES):
        b = c // CORES_PER_SAMPLE
        q = c % CORES_PER_SAMPLE
        sl = slice(q * CHUNK, (q + 1) * CHUNK)
        zp = y_pred[b].reshape(C, N)
        in_maps.append({
            "z0": np.ascontiguousarray(zp[0, sl]).reshape(P, F),
            "z1": np.ascontiguousarray(zp[1, sl]).reshape(P, F),
            "yb": np.ascontiguousarray(
                y[b, 0].reshape(N)[sl]).astype(np.uint8).reshape(P, F),
            "vor": np.ascontiguousarray(
                voronoi[b].reshape(N)[sl]).astype(np.uint8).reshape(P, F),
            "negk": negk,
            "sigb": sigb,
        })

    res = bass_utils.run_bass_kernel_spmd(
        nc, in_maps, core_ids=list(range(NCORES)), trace=TRACE,
    )
    kernel.last_results = res

    # ---- host-side gather/unshard: combine per-core partials ----
    R = np.zeros((B, KMAX + 2), dtype=np.float64)
    Tm = np.zeros((B, KMAX + 2), dtype=np.float64)
    Fm = np.zeros((B, KMAX + 2), dtype=np.float64)
    for c in range(NCORES):
        b = c // CORES_PER_SAMPLE
        out = np.asarray(res.results[c]["out"], dtype=np.float64)
        R[b, 1:KMAX + 1] += out[:, 0:KMAX].sum(axis=0)
        Tm[b, 1:KMAX + 1] += out[:, KMAX:2 * KMAX].sum(axis=0)
        Fm[b, 1:KMAX + 1] += out[:, 2 * KMAX:3 * KMAX].sum(axis=0)

    scores = []
    for b in range(B):
        cnt = np.zeros(KMAX + 2)
        T = np.zeros(KMAX + 2)          # reconstructed T_k, T_65 = 0
        for k in range(KMAX, 0, -1):
            if k in DVE_BINS:
                T[k] = Tm[b, k]
                cnt[k] = T[k] - T[k + 1]
            else:
                cnt[k] = 2.0 * (Fm[b, k] - T[k + 1])
                T[k] = T[k + 1] + cnt[k]
        k = np.arange(1, KMAX + 1)
        M = R[b, k] - R[b, k + 1]
        inter = M - T[k + 1]
        cntk = cnt[k]
        # counts are integers; snap to kill sigmoid-chain noise
        cntk = np.round(cntk)
        dice = (2.0 * inter + EPS) / (2.0 * cntk + EPS)
        present = cntk > 0
        n_present = max(present.sum(), 1)
        scores.append(np.where(present, dice, 0.0).sum() / n_present)

    return np.float32(np.mean(scores))


# revision 8
# speedup vs baseline: 1.0485x; 1.0485x over previous
"""Trainium2 Bass kernel for nn_CCMetrics (connected-component soft-Dice).

Math
----
Reference per sample: probs = softmax(y_pred, ch axis 1) with C=2 channels,
one-hot labels y in {0,1}.  Per-voxel channel sums collapse:
  psum_v = tsum_v = 1          (softmax / one-hot sum to 1 over channels)
  inter_v = probs[true_ch] = sigmoid((2y-1) * (z1 - z0))
So per segment id k (voronoi component, 0..64):
  inter_k = sum of sigmoid values over voxels with id k
  cnt_k   = voxel count with id k
  dice_k  = (2*inter_k + eps) / (2*cnt_k + eps)
  score   = mean over present k in 1..64;  output = mean over batch.

Device algorithm (per core, data-parallel over 4M voxels / 8 cores)
------------------------------------------------------------------
Build two packed streams per voxel (id g, value v = sigmoid(...)):
  z  = g + 0.5                  (exact half-integers)
  x' = g + 0.5 + v              (value stream, thresholds at k+0.5)
Cumulative families, one instruction per bin k (per-partition accumulate):
  R_k = sum relu(x' - (k+0.5))           [ACT Relu + bias + accum]
  T_k = #{x' >= k+0.5} = #{g >= k}       [DVE tensor_scalar is_ge + accum]
  F_k = sum sigmoid(30*(z-(k+0.5)))      [ACT Sigmoid + bias + accum]
        = 0.5*cnt_k + T_{k+1}   (exact to ~1e-13: args are multiples of 30)
Recovery (host, float64):  M_k = R_k - R_{k+1} = inter_k + T_{k+1};
walking k = 64..1 with T_65 = 0: exact T anchors from DVE bins, F bins give
cnt_k = 2*(F_k - T_{k+1}).  ACT pipelines accumulate passes at ~1.3 us while
DVE accumulate passes have a ~4.3 us drain period, so ACT takes the relu
family plus most count bins (sigmoid) and DVE takes preprocessing plus a
spread subset of exact-count anchor bins.
"""

import os
import sys

import numpy as np

for _p in ("/opt/trn_rl_repo",):
    if os.path.isdir(_p) and _p not in sys.path:
        sys.path.insert(0, _p)

from concourse import bacc, bass, mybir, tile  # noqa: E402
from concourse import bass_utils  # noqa: E402

NUM_COMP = 64
EPS = 1e-5
B, C, H, W, D = 2, 2, 128, 128, 128
N = H * W * D
NCORES = 8
CORES_PER_SAMPLE = NCORES // B
CHUNK = N // CORES_PER_SAMPLE
P = 128
F = CHUNK // P
KMAX = NUM_COMP

# Exact-count anchor bins computed on DVE (tensor_scalar is_ge + accum).
# Spread so that sigmoid-chain reconstruction segments stay short.
_nd = int(os.environ.get("CC_ND", "27"))
if _nd >= KMAX:
    DVE_BINS = frozenset(range(1, KMAX + 1))
else:
    # evenly spread anchors from k=KMAX downward
    _step = max(1, round(KMAX / max(_nd, 1)))
    DVE_BINS = frozenset(
        k for k in range(KMAX, 0, -_step)
    ) | {KMAX}
    DVE_BINS = frozenset(sorted(DVE_BINS, reverse=True)[:max(_nd, 1)])
TRACE = False

_prog_cache = {}


def _build_program():
    nc = bacc.Bacc(
        "TRN2",
        target_bir_lowering=False,
        debug=False,
        enable_asserts=False,
        num_devices=NCORES,
    )
    f32 = mybir.dt.float32
    u8 = mybir.dt.uint8

    z0_d = nc.dram_tensor("z0", [P, F], f32, kind="ExternalInput").ap()
    z1_d = nc.dram_tensor("z1", [P, F], f32, kind="ExternalInput").ap()
    y_d = nc.dram_tensor("yb", [P, F], u8, kind="ExternalInput").ap()
    g_d = nc.dram_tensor("vor", [P, F], u8, kind="ExternalInput").ap()
    # bias constants: col j (j=0..63): -(j+1.5) for relu; col 64: 0.0
    negk_d = nc.dram_tensor("negk", [P, KMAX + 1], f32, kind="ExternalInput").ap()
    # sigmoid bias constants: col j: -30*(j+1.5)
    sigb_d = nc.dram_tensor("sigb", [P, KMAX], f32, kind="ExternalInput").ap()
    out_d = nc.dram_tensor("out", [P, 3 * KMAX], f32, kind="ExternalOutput").ap()

    Alu = mybir.AluOpType
    Act = mybir.ActivationFunctionType

    with tile.TileContext(nc) as tc:
        with tc.tile_pool(name="main", bufs=1) as pool:
            z0 = pool.tile([P, F], f32)
            z1 = pool.tile([P, F], f32)
            yt = pool.tile([P, F], u8)
            gt = pool.tile([P, F], u8)
            negk = pool.tile([P, KMAX + 1], f32)
            sigb = pool.tile([P, KMAX], f32)
            nc.sync.dma_start(out=z0[:], in_=z0_d[:])
            nc.sync.dma_start(out=z1[:], in_=z1_d[:])
            nc.sync.dma_start(out=yt[:], in_=y_d[:])
            nc.sync.dma_start(out=gt[:], in_=g_d[:])
            nc.sync.dma_start(out=negk[:], in_=negk_d[:])
            nc.sync.dma_start(out=sigb[:], in_=sigb_d[:])

            # ---- preprocessing (DVE + one ACT sigmoid) ----
            s = pool.tile([P, F], f32)
            nc.vector.tensor_sub(s[:], z1[:], z0[:])
            yf = pool.tile([P, F], f32)
            nc.vector.tensor_scalar(
                out=yf[:], in0=yt[:], scalar1=2.0, scalar2=-1.0,
                op0=Alu.mult, op1=Alu.add,
            )
            t = pool.tile([P, F], f32)
            nc.vector.tensor_mul(t[:], s[:], yf[:])
            v = pool.tile([P, F], f32)
            nc.scalar.activation(
                out=v[:], in_=t[:], func=Act.Sigmoid,
                bias=negk[:, KMAX:KMAX + 1], scale=1.0,  # bias 0.0
            )
            zt = pool.tile([P, F], f32)
            nc.vector.tensor_scalar(
                out=zt[:], in0=gt[:], scalar1=0.5, scalar2=None, op0=Alu.add,
            )
            x = pool.tile([P, F], f32)
            nc.vector.tensor_add(x[:], v[:], zt[:])

            racc = pool.tile([P, KMAX], f32)
            tacc = pool.tile([P, KMAX], f32)
            facc = pool.tile([P, KMAX], f32)
            trash_a = pool.tile([P, F], f32)
            trash_d = pool.tile([P, F], f32)

            trash_s = pool.tile([P, F], f32)
            # sigmoid count block first (same ACT table as the v pass; only
            # needs zt, so it runs while x is still being built)
            for k in range(1, KMAX + 1):
                j = k - 1
                if k not in DVE_BINS:
                    nc.scalar.activation(
                        out=trash_s[:], in_=zt[:], func=Act.Sigmoid,
                        bias=sigb[:, j:j + 1], scale=30.0,
                        accum_out=facc[:, j:j + 1],
                    )
            # exact count anchors on DVE
            for k in range(1, KMAX + 1):
                j = k - 1
                if k in DVE_BINS:
                    nc.vector.tensor_scalar(
                        out=trash_d[:], in0=x[:], scalar1=float(k) + 0.5,
                        scalar2=None, op0=Alu.is_ge, op1=Alu.add,
                        accum_out=tacc[:, j:j + 1],
                    )
            # relu value block on ACT (one activation-table switch total)
            for k in range(1, KMAX + 1):
                j = k - 1
                nc.scalar.activation(
                    out=trash_a[:], in_=x[:], func=Act.Relu,
                    bias=negk[:, j:j + 1], scale=1.0,
                    accum_out=racc[:, j:j + 1],
                )

            nc.sync.dma_start(out=out_d[:, 0:KMAX], in_=racc[:])
            nc.sync.dma_start(out=out_d[:, KMAX:2 * KMAX], in_=tacc[:])
            nc.sync.dma_start(out=out_d[:, 2 * KMAX:3 * KMAX], in_=facc[:])

    nc.compile()
    return nc


def _get_program():
    key = ("prog", tuple(sorted(DVE_BINS)))
    if key not in _prog_cache:
        _prog_cache[key] = _build_program()
    return _prog_cache[key]


def _consts():
    negk = np.concatenate(
        [-(np.arange(1, KMAX + 1, dtype=np.float32) + 0.5), np.zeros(1, np.float32)])
    sigb = -30.0 * (np.arange(1, KMAX + 1, dtype=np.float32) + 0.5)
    return (np.broadcast_to(negk, (P, KMAX + 1)).copy(),
            np.broadcast_to(sigb, (P, KMAX)).copy())


def kernel(y_pred: np.ndarray, y: np.ndarray, voronoi: np.ndarray) -> np.ndarray:
    y_pred = np.asarray(y_pred, dtype=np.float32)
    y = np.asarray(y)
    voronoi = np.asarray(voronoi)

    nc = _get_program()
    negk, sigb = _consts()

    in_maps = []
    for c in range(NCORES):
        b = c // CORES_PER_SAMPLE
        q = c % CORES_PER_SAMPLE
        sl = slice(q * CHUNK, (q + 1) * CHUNK)
        zp = y_pred[b].reshape(C, N)
        in_maps.append({
            "z0": np.ascontiguousarray(zp[0, sl]).reshape(P, F),
            "z1": np.ascontiguousarray(zp[1, sl]).reshape(P, F),
            "yb": np.ascontiguousarray(
                y[b, 0].reshape(N)[sl]).astype(np.uint8).reshape(P, F),
            "vor": np.ascontiguousarray(
                voronoi[b].reshape(N)[sl]).astype(np.uint8).reshape(P, F),
            "negk": negk,
            "sigb": sigb,
        })

    res = bass_utils.run_bass_kernel_spmd(
        nc, in_maps, core_ids=list(range(NCORES)), trace=TRACE,
    )
    kernel.last_results = res

    # ---- host-side gather/unshard: combine per-core partials ----
    R = np.zeros((B, KMAX + 2), dtype=np.float64)
    Tm = np.zeros((B, KMAX + 2), dtype=np.float64)
    Fm = np.zeros((B, KMAX + 2), dtype=np.float64)
    for c in range(NCORES):
        b = c // CORES_PER_SAMPLE
        out = np.asarray(res.results[c]["out"], dtype=np.float64)
        R[b, 1:KMAX + 1] += out[:, 0:KMAX].sum(axis=0)
        Tm[b, 1:KMAX + 1] += out[:, KMAX:2 * KMAX].sum(axis=0)
        Fm[b, 1:KMAX + 1] += out[:, 2 * KMAX:3 * KMAX].sum(axis=0)

    scores = []
    for b in range(B):
        cnt = np.zeros(KMAX + 2)
        T = np.zeros(KMAX + 2)          # reconstructed T_k, T_65 = 0
        for k in range(KMAX, 0, -1):
            if k in DVE_BINS:
                T[k] = Tm[b, k]
                cnt[k] = T[k] - T[k + 1]
            else:
                cnt[k] = 2.0 * (Fm[b, k] - T[k + 1])
                T[k] = T[k + 1] + cnt[k]
        k = np.arange(1, KMAX + 1)
        M = R[b, k] - R[b, k + 1]
        inter = M - T[k + 1]
        cntk = cnt[k]
        # counts are integers; snap to kill sigmoid-chain noise
        cntk = np.round(cntk)
        dice = (2.0 * inter + EPS) / (2.0 * cntk + EPS)
        present = cntk > 0
        n_present = max(present.sum(), 1)
        scores.append(np.where(present, dice, 0.0).sum() / n_present)

    return np.float32(np.mean(scores))


# revision 9
# speedup vs baseline: 1.4054x; 1.3404x over previous
"""Trainium2 Bass kernel for nn_CCMetrics (connected-component soft-Dice).

Math
----
Reference per sample: probs = softmax(y_pred, ch axis 1) with C=2 channels,
one-hot labels y in {0,1}.  Per-voxel channel sums collapse:
  psum_v = tsum_v = 1          (softmax / one-hot sum to 1 over channels)
  inter_v = probs[true_ch] = sigmoid((2y-1) * (z1 - z0))
So per segment id k (voronoi component, 0..64):
  inter_k = sum of sigmoid values over voxels with id k
  cnt_k   = voxel count with id k
  dice_k  = (2*inter_k + eps) / (2*cnt_k + eps)
  score   = mean over present k in 1..64;  output = mean over batch.

Device algorithm (per core, data-parallel over 4M voxels / 8 cores)
------------------------------------------------------------------
Build two packed streams per voxel (id g, value v = sigmoid(...)):
  z  = g + 0.5                  (exact half-integers)
  x' = g + 0.5 + v              (value stream, thresholds at k+0.5)
Cumulative families, one instruction per bin k (per-partition accumulate):
  R_k = sum relu(x' - (k+0.5))           [ACT Relu + bias + accum]
  T_k = #{x' >= k+0.5} = #{g >= k}       [DVE tensor_scalar is_ge + accum]
  F_k = sum sigmoid(30*(z-(k+0.5)))      [ACT Sigmoid + bias + accum]
        = 0.5*cnt_k + T_{k+1}   (exact to ~1e-13: args are multiples of 30)
Recovery (host, float64):  M_k = R_k - R_{k+1} = inter_k + T_{k+1};
walking k = 64..1 with T_65 = 0: exact T anchors from DVE bins, F bins give
cnt_k = 2*(F_k - T_{k+1}).  ACT pipelines accumulate passes at ~1.3 us while
DVE accumulate passes have a ~4.3 us drain period, so ACT takes the relu
family plus most count bins (sigmoid) and DVE takes preprocessing plus a
spread subset of exact-count anchor bins.
"""

import os
import sys

import numpy as np

for _p in ("/opt/trn_rl_repo",):
    if os.path.isdir(_p) and _p not in sys.path:
        sys.path.insert(0, _p)

from concourse import bacc, bass, mybir, tile  # noqa: E402
from concourse import bass_utils  # noqa: E402

NUM_COMP = 64
EPS = 1e-5
B, C, H, W, D = 2, 2, 128, 128, 128
N = H * W * D
NCORES = 8
CORES_PER_SAMPLE = NCORES // B
CHUNK = N // CORES_PER_SAMPLE
P = 128
F = CHUNK // P
KMAX = NUM_COMP

# Exact-count anchor bins computed on DVE (tensor_scalar is_ge + accum).
# Spread so that sigmoid-chain reconstruction segments stay short.
_nd = int(os.environ.get("CC_ND", "57"))
if _nd >= KMAX:
    DVE_BINS = frozenset(range(1, KMAX + 1))
else:
    # evenly spread anchors from k=KMAX downward
    _step = max(1, round(KMAX / max(_nd, 1)))
    DVE_BINS = frozenset(
        k for k in range(KMAX, 0, -_step)
    ) | {KMAX}
    DVE_BINS = frozenset(sorted(DVE_BINS, reverse=True)[:max(_nd, 1)])
TRACE = False

_prog_cache = {}


def _build_program():
    nc = bacc.Bacc(
        "TRN2",
        target_bir_lowering=False,
        debug=False,
        enable_asserts=False,
        num_devices=NCORES,
    )
    f32 = mybir.dt.float32
    u8 = mybir.dt.uint8

    z0_d = nc.dram_tensor("z0", [P, F], f32, kind="ExternalInput").ap()
    z1_d = nc.dram_tensor("z1", [P, F], f32, kind="ExternalInput").ap()
    y_d = nc.dram_tensor("yb", [P, F], u8, kind="ExternalInput").ap()
    g_d = nc.dram_tensor("vor", [P, F], u8, kind="ExternalInput").ap()
    # bias constants: col j (j=0..63): -(j+1.5) for relu; col 64: 0.0
    negk_d = nc.dram_tensor("negk", [P, KMAX + 1], f32, kind="ExternalInput").ap()
    # sigmoid bias constants: col j: -30*(j+1.5)
    sigb_d = nc.dram_tensor("sigb", [P, KMAX], f32, kind="ExternalInput").ap()
    out_d = nc.dram_tensor("out", [P, 3 * KMAX], f32, kind="ExternalOutput").ap()

    Alu = mybir.AluOpType
    Act = mybir.ActivationFunctionType

    with tile.TileContext(nc) as tc:
        with tc.tile_pool(name="main", bufs=1) as pool:
            z0 = pool.tile([P, F], f32)
            z1 = pool.tile([P, F], f32)
            yt = pool.tile([P, F], u8)
            gt = pool.tile([P, F], u8)
            negk = pool.tile([P, KMAX + 1], f32)
            sigb = pool.tile([P, KMAX], f32)
            nc.sync.dma_start(out=z0[:], in_=z0_d[:])
            nc.sync.dma_start(out=z1[:], in_=z1_d[:])
            nc.sync.dma_start(out=yt[:], in_=y_d[:])
            nc.sync.dma_start(out=gt[:], in_=g_d[:])
            nc.sync.dma_start(out=negk[:], in_=negk_d[:])
            nc.sync.dma_start(out=sigb[:], in_=sigb_d[:])

            # ---- preprocessing (DVE + one ACT sigmoid) ----
            s = pool.tile([P, F], f32)
            nc.vector.tensor_sub(s[:], z1[:], z0[:])
            yf = pool.tile([P, F], f32)
            nc.vector.tensor_scalar(
                out=yf[:], in0=yt[:], scalar1=2.0, scalar2=-1.0,
                op0=Alu.mult, op1=Alu.add,
            )
            t = pool.tile([P, F], f32)
            nc.vector.tensor_mul(t[:], s[:], yf[:])
            v = pool.tile([P, F], f32)
            nc.scalar.activation(
                out=v[:], in_=t[:], func=Act.Sigmoid,
                bias=negk[:, KMAX:KMAX + 1], scale=1.0,  # bias 0.0
            )
            zt = pool.tile([P, F], f32)
            nc.vector.tensor_scalar(
                out=zt[:], in0=gt[:], scalar1=0.5, scalar2=None, op0=Alu.add,
            )
            x = pool.tile([P, F], f32)
            nc.vector.tensor_add(x[:], v[:], zt[:])

            racc = pool.tile([P, KMAX], f32)
            tacc = pool.tile([P, KMAX], f32)
            facc = pool.tile([P, KMAX], f32)
            trash_a = pool.tile([P, F], f32)
            trash_d = pool.tile([P, F], f32)

            trash_s = pool.tile([P, F], f32)
            # sigmoid count block first (same ACT table as the v pass; only
            # needs zt, so it runs while x is still being built)
            for k in range(1, KMAX + 1):
                j = k - 1
                if k not in DVE_BINS:
                    nc.scalar.activation(
                        out=trash_s[:], in_=zt[:], func=Act.Sigmoid,
                        bias=sigb[:, j:j + 1], scale=30.0,
                        accum_out=facc[:, j:j + 1],
                    )
            # exact count anchors on DVE
            for k in range(1, KMAX + 1):
                j = k - 1
                if k in DVE_BINS:
                    nc.vector.tensor_scalar(
                        out=trash_d[:], in0=x[:], scalar1=float(k) + 0.5,
                        scalar2=None, op0=Alu.is_ge, op1=Alu.add,
                        accum_out=tacc[:, j:j + 1],
                    )
            # relu value block on ACT (one activation-table switch total)
            for k in range(1, KMAX + 1):
                j = k - 1
                nc.scalar.activation(
                    out=trash_a[:], in_=x[:], func=Act.Relu,
                    bias=negk[:, j:j + 1], scale=1.0,
                    accum_out=racc[:, j:j + 1],
                )

            nc.sync.dma_start(out=out_d[:, 0:KMAX], in_=racc[:])
            nc.sync.dma_start(out=out_d[:, KMAX:2 * KMAX], in_=tacc[:])
            nc.sync.dma_start(out=out_d[:, 2 * KMAX:3 * KMAX], in_=facc[:])

    nc.compile()
    return nc


def _get_program():
    key = ("prog", tuple(sorted(DVE_BINS)))
    if key not in _prog_cache:
        _prog_cache[key] = _build_program()
    return _prog_cache[key]


def _consts():
    negk = np.concatenate(
        [-(np.arange(1, KMAX + 1, dtype=np.float32) + 0.5), np.zeros(1, np.float32)])
    sigb = -30.0 * (np.arange(1, KMAX + 1, dtype=np.float32) + 0.5)
    return (np.broadcast_to(negk, (P, KMAX + 1)).copy(),
            np.broadcast_to(sigb, (P, KMAX)).copy())


def kernel(y_pred: np.ndarray, y: np.ndarray, voronoi: np.ndarray) -> np.ndarray:
    y_pred = np.asarray(y_pred, dtype=np.float32)
    y = np.asarray(y)
    voronoi = np.asarray(voronoi)

    nc = _get_program()
    negk, sigb = _consts()

    in_maps = []
    for c in range(NCORES):
        b = c // CORES_PER_SAMPLE
        q = c % CORES_PER_SAMPLE
        sl = slice(q * CHUNK, (q + 1) * CHUNK)
        zp = y_pred[b].reshape(C, N)
        in_maps.append({
            "z0": np.ascontiguousarray(zp[0, sl]).reshape(P, F),
            "z1": np.ascontiguousarray(zp[1, sl]).reshape(P, F),
            "yb": np.ascontiguousarray(
                y[b, 0].reshape(N)[sl]).astype(np.uint8).reshape(P, F),
            "vor": np.ascontiguousarray(
                voronoi[b].reshape(N)[sl]).astype(np.uint8).reshape(P, F),
            "negk": negk,
            "sigb": sigb,
        })

    res = bass_utils.run_bass_kernel_spmd(
        nc, in_maps, core_ids=list(range(NCORES)), trace=TRACE,
    )
    kernel.last_results = res

    # ---- host-side gather/unshard: combine per-core partials ----
    R = np.zeros((B, KMAX + 2), dtype=np.float64)
    Tm = np.zeros((B, KMAX + 2), dtype=np.float64)
    Fm = np.zeros((B, KMAX + 2), dtype=np.float64)
    for c in range(NCORES):
        b = c // CORES_PER_SAMPLE
        out = np.asarray(res.results[c]["out"], dtype=np.float64)
        R[b, 1:KMAX + 1] += out[:, 0:KMAX].sum(axis=0)
        Tm[b, 1:KMAX + 1] += out[:, KMAX:2 * KMAX].sum(axis=0)
        Fm[b, 1:KMAX + 1] += out[:, 2 * KMAX:3 * KMAX].sum(axis=0)

    scores = []
    for b in range(B):
        cnt = np.zeros(KMAX + 2)
        T = np.zeros(KMAX + 2)          # reconstructed T_k, T_65 = 0
        for k in range(KMAX, 0, -1):
            if k in DVE_BINS:
                T[k] = Tm[b, k]
                cnt[k] = T[k] - T[k + 1]
            else:
                cnt[k] = 2.0 * (Fm[b, k] - T[k + 1])
                T[k] = T[k + 1] + cnt[k]
        k = np.arange(1, KMAX + 1)
        M = R[b, k] - R[b, k + 1]
        inter = M - T[k + 1]
        cntk = cnt[k]
        # counts are integers; snap to kill sigmoid-chain noise
        cntk = np.round(cntk)
        dice = (2.0 * inter + EPS) / (2.0 * cntk + EPS)
        present = cntk > 0
        n_present = max(present.sum(), 1)
        scores.append(np.where(present, dice, 0.0).sum() / n_present)

    return np.float32(np.mean(scores))


# revision 10
# speedup vs baseline: 1.4294x; 1.0170x over previous
"""Trainium2 Bass kernel for nn_CCMetrics (connected-component soft-Dice).

Math
----
Reference per sample: probs = softmax(y_pred, ch axis 1) with C=2 channels,
one-hot labels y in {0,1}.  Per-voxel channel sums collapse:
  psum_v = tsum_v = 1          (softmax / one-hot sum to 1 over channels)
  inter_v = probs[true_ch] = sigmoid((2y-1) * (z1 - z0))
So per segment id k (voronoi component, 0..64):
  inter_k = sum of sigmoid values over voxels with id k
  cnt_k   = voxel count with id k
  dice_k  = (2*inter_k + eps) / (2*cnt_k + eps)
  score   = mean over present k in 1..64;  output = mean over batch.

Device algorithm (per core, data-parallel over 4M voxels / 8 cores)
------------------------------------------------------------------
Build two packed streams per voxel (id g, value v = sigmoid(...)):
  z  = g + 0.5                  (exact half-integers)
  x' = g + 0.5 + v              (value stream, thresholds at k+0.5)
Cumulative families, one instruction per bin k (per-partition accumulate):
  R_k = sum relu(x' - (k+0.5))           [ACT Relu + bias + accum]
  T_k = #{x' >= k+0.5} = #{g >= k}       [DVE tensor_scalar is_ge + accum]
  F_k = sum sigmoid(30*(z-(k+0.5)))      [ACT Sigmoid + bias + accum]
        = 0.5*cnt_k + T_{k+1}   (exact to ~1e-13: args are multiples of 30)
Recovery (host, float64):  M_k = R_k - R_{k+1} = inter_k + T_{k+1};
walking k = 64..1 with T_65 = 0: exact T anchors from DVE bins, F bins give
cnt_k = 2*(F_k - T_{k+1}).  ACT pipelines accumulate passes at ~1.3 us while
DVE accumulate passes have a ~4.3 us drain period, so ACT takes the relu
family plus most count bins (sigmoid) and DVE takes preprocessing plus a
spread subset of exact-count anchor bins.
"""

import os
import sys

import numpy as np

for _p in ("/opt/trn_rl_repo",):
    if os.path.isdir(_p) and _p not in sys.path:
        sys.path.insert(0, _p)

from concourse import bacc, bass, mybir, tile  # noqa: E402
from concourse import bass_utils  # noqa: E402

NUM_COMP = 64
EPS = 1e-5
B, C, H, W, D = 2, 2, 128, 128, 128
N = H * W * D
NCORES = 8
CORES_PER_SAMPLE = NCORES // B
CHUNK = N // CORES_PER_SAMPLE
P = 128
F = CHUNK // P
KMAX = NUM_COMP

# Exact-count anchor bins computed on DVE (tensor_scalar is_ge + accum).
# Spread so that sigmoid-chain reconstruction segments stay short.
_nd = int(os.environ.get("CC_ND", "57"))
if _nd >= KMAX:
    DVE_BINS = frozenset(range(1, KMAX + 1))
else:
    # evenly spread anchors from k=KMAX downward
    _step = max(1, round(KMAX / max(_nd, 1)))
    DVE_BINS = frozenset(
        k for k in range(KMAX, 0, -_step)
    ) | {KMAX}
    DVE_BINS = frozenset(sorted(DVE_BINS, reverse=True)[:max(_nd, 1)])
TRACE = False

_prog_cache = {}


def _build_program():
    nc = bacc.Bacc(
        "TRN2",
        target_bir_lowering=False,
        debug=False,
        enable_asserts=False,
        num_devices=NCORES,
    )
    f32 = mybir.dt.float32
    u8 = mybir.dt.uint8

    z0_d = nc.dram_tensor("z0", [P, F], f32, kind="ExternalInput").ap()
    z1_d = nc.dram_tensor("z1", [P, F], f32, kind="ExternalInput").ap()
    y_d = nc.dram_tensor("yb", [P, F], u8, kind="ExternalInput").ap()
    g_d = nc.dram_tensor("vor", [P, F], u8, kind="ExternalInput").ap()
    # bias constants: col j (j=0..63): -(j+1.5) for relu; col 64: 0.0
    negk_d = nc.dram_tensor("negk", [P, KMAX + 1], f32, kind="ExternalInput").ap()
    # sigmoid bias constants: col j: -30*(j+1.5)
    sigb_d = nc.dram_tensor("sigb", [P, KMAX], f32, kind="ExternalInput").ap()
    out_d = nc.dram_tensor("out", [P, 3 * KMAX], f32, kind="ExternalOutput").ap()

    Alu = mybir.AluOpType
    Act = mybir.ActivationFunctionType

    with tile.TileContext(nc) as tc:
        with tc.tile_pool(name="main", bufs=1) as pool:
            z0 = pool.tile([P, F], f32)
            z1 = pool.tile([P, F], f32)
            yt = pool.tile([P, F], u8)
            gt = pool.tile([P, F], u8)
            negk = pool.tile([P, KMAX + 1], f32)
            sigb = pool.tile([P, KMAX], f32)
            nc.sync.dma_start(out=z0[:], in_=z0_d[:])
            nc.sync.dma_start(out=z1[:], in_=z1_d[:])
            nc.sync.dma_start(out=yt[:], in_=y_d[:])
            nc.sync.dma_start(out=gt[:], in_=g_d[:])
            nc.sync.dma_start(out=negk[:], in_=negk_d[:])
            nc.sync.dma_start(out=sigb[:], in_=sigb_d[:])

            # ---- preprocessing (DVE) ----
            zt = pool.tile([P, F], f32)
            nc.vector.tensor_scalar(
                out=zt[:], in0=gt[:], scalar1=0.5, scalar2=None, op0=Alu.add,
            )
            s = pool.tile([P, F], f32)
            nc.vector.tensor_sub(s[:], z1[:], z0[:])
            yf = pool.tile([P, F], f32)
            nc.vector.tensor_scalar(
                out=yf[:], in0=yt[:], scalar1=2.0, scalar2=-1.0,
                op0=Alu.mult, op1=Alu.add,
            )
            t = pool.tile([P, F], f32)
            nc.vector.tensor_mul(t[:], s[:], yf[:])

            racc = pool.tile([P, KMAX], f32)
            tacc = pool.tile([P, KMAX], f32)
            facc = pool.tile([P, KMAX], f32)
            trash_a = pool.tile([P, F], f32)
            trash_d = pool.tile([P, F], f32)
            trash_s = pool.tile([P, F], f32)

            sig_bins = [k for k in range(1, KMAX + 1) if k not in DVE_BINS]

            # a few sigmoid count passes first: they only need zt, so ACT
            # starts ~3us in while the DVE preprocessing chain runs
            def emit_sig(k):
                j = k - 1
                nc.scalar.activation(
                    out=trash_s[:], in_=zt[:], func=Act.Sigmoid,
                    bias=sigb[:, j:j + 1], scale=30.0,
                    accum_out=facc[:, j:j + 1],
                )

            head = sig_bins[:3]
            for k in head:
                emit_sig(k)
            v = pool.tile([P, F], f32)
            nc.scalar.activation(
                out=v[:], in_=t[:], func=Act.Sigmoid,
                bias=negk[:, KMAX:KMAX + 1], scale=1.0,  # bias 0.0
            )
            for k in sig_bins[3:]:
                emit_sig(k)
            x = pool.tile([P, F], f32)
            nc.vector.tensor_add(x[:], v[:], zt[:])
            # exact count anchors on DVE
            for k in range(1, KMAX + 1):
                j = k - 1
                if k in DVE_BINS:
                    nc.vector.tensor_scalar(
                        out=trash_d[:], in0=x[:], scalar1=float(k) + 0.5,
                        scalar2=None, op0=Alu.is_ge, op1=Alu.add,
                        accum_out=tacc[:, j:j + 1],
                    )
            # relu value block on ACT (one activation-table switch total)
            for k in range(1, KMAX + 1):
                j = k - 1
                nc.scalar.activation(
                    out=trash_a[:], in_=x[:], func=Act.Relu,
                    bias=negk[:, j:j + 1], scale=1.0,
                    accum_out=racc[:, j:j + 1],
                )

            nc.sync.dma_start(out=out_d[:, 0:KMAX], in_=racc[:])
            nc.sync.dma_start(out=out_d[:, KMAX:2 * KMAX], in_=tacc[:])
            nc.sync.dma_start(out=out_d[:, 2 * KMAX:3 * KMAX], in_=facc[:])

    nc.compile()
    return nc


def _get_program():
    key = ("prog", tuple(sorted(DVE_BINS)))
    if key not in _prog_cache:
        _prog_cache[key] = _build_program()
    return _prog_cache[key]


def _consts():
    negk = np.concatenate(
        [-(np.arange(1, KMAX + 1, dtype=np.float32) + 0.5), np.zeros(1, np.float32)])
    sigb = -30.0 * (np.arange(1, KMAX + 1, dtype=np.float32) + 0.5)
    return (np.broadcast_to(negk, (P, KMAX + 1)).copy(),
            np.broadcast_to(sigb, (P, KMAX)).copy())


def kernel(y_pred: np.ndarray, y: np.ndarray, voronoi: np.ndarray) -> np.ndarray:
    y_pred = np.asarray(y_pred, dtype=np.float32)
    y = np.asarray(y)
    voronoi = np.asarray(voronoi)

    nc = _get_program()
    negk, sigb = _consts()

    in_maps = []
    for c in range(NCORES):
        b = c // CORES_PER_SAMPLE
        q = c % CORES_PER_SAMPLE
        sl = slice(q * CHUNK, (q + 1) * CHUNK)
        zp = y_pred[b].reshape(C, N)
        in_maps.append({
            "z0": np.ascontiguousarray(zp[0, sl]).reshape(P, F),
            "z1": np.ascontiguousarray(zp[1, sl]).reshape(P, F),
            "yb": np.ascontiguousarray(
                y[b, 0].reshape(N)[sl]).astype(np.uint8).reshape(P, F),
            "vor": np.ascontiguousarray(
                voronoi[b].reshape(N)[sl]).astype(np.uint8).reshape(P, F),
            "negk": negk,
            "sigb": sigb,
        })

    res = bass_utils.run_bass_kernel_spmd(
        nc, in_maps, core_ids=list(range(NCORES)), trace=TRACE,
    )
    kernel.last_results = res

    # ---- host-side gather/unshard: combine per-core partials ----
    R = np.zeros((B, KMAX + 2), dtype=np.float64)
    Tm = np.zeros((B, KMAX + 2), dtype=np.float64)
    Fm = np.zeros((B, KMAX + 2), dtype=np.float64)
    for c in range(NCORES):
        b = c // CORES_PER_SAMPLE
        out = np.asarray(res.results[c]["out"], dtype=np.float64)
        R[b, 1:KMAX + 1] += out[:, 0:KMAX].sum(axis=0)
        Tm[b, 1:KMAX + 1] += out[:, KMAX:2 * KMAX].sum(axis=0)
        Fm[b, 1:KMAX + 1] += out[:, 2 * KMAX:3 * KMAX].sum(axis=0)

    scores = []
    for b in range(B):
        cnt = np.zeros(KMAX + 2)
        T = np.zeros(KMAX + 2)          # reconstructed T_k, T_65 = 0
        for k in range(KMAX, 0, -1):
            if k in DVE_BINS:
                T[k] = Tm[b, k]
                cnt[k] = T[k] - T[k + 1]
            else:
                cnt[k] = 2.0 * (Fm[b, k] - T[k + 1])
                T[k] = T[k + 1] + cnt[k]
        k = np.arange(1, KMAX + 1)
        M = R[b, k] - R[b, k + 1]
        inter = M - T[k + 1]
        cntk = cnt[k]
        # counts are integers; snap to kill sigmoid-chain noise
        cntk = np.round(cntk)
        dice = (2.0 * inter + EPS) / (2.0 * cntk + EPS)
        present = cntk > 0
        n_present = max(present.sum(), 1)
        scores.append(np.where(present, dice, 0.0).sum() / n_present)

    return np.float32(np.mean(scores))


# revision 11
# speedup vs baseline: 1.4427x; 1.0093x over previous
"""Trainium2 Bass kernel for nn_CCMetrics (connected-component soft-Dice).

Math
----
Reference per sample: probs = softmax(y_pred, ch axis 1) with C=2 channels,
one-hot labels y in {0,1}.  Per-voxel channel sums collapse:
  psum_v = tsum_v = 1          (softmax / one-hot sum to 1 over channels)
  inter_v = probs[true_ch] = sigmoid((2y-1) * (z1 - z0))
So per segment id k (voronoi component, 0..64):
  inter_k = sum of sigmoid values over voxels with id k
  cnt_k   = voxel count with id k
  dice_k  = (2*inter_k + eps) / (2*cnt_k + eps)
  score   = mean over present k in 1..64;  output = mean over batch.

Device algorithm (per core, data-parallel over 4M voxels / 8 cores)
------------------------------------------------------------------
Build two packed streams per voxel (id g, value v = sigmoid(...)):
  z  = g + 0.5                  (exact half-integers)
  x' = g + 0.5 + v              (value stream, thresholds at k+0.5)
Cumulative families, one instruction per bin k (per-partition accumulate):
  R_k = sum relu(x' - (k+0.5))           [ACT Relu + bias + accum]
  T_k = #{x' >= k+0.5} = #{g >= k}       [DVE tensor_scalar is_ge + accum]
  F_k = sum sigmoid(30*(z-(k+0.5)))      [ACT Sigmoid + bias + accum]
        = 0.5*cnt_k + T_{k+1}   (exact to ~1e-13: args are multiples of 30)
Recovery (host, float64):  M_k = R_k - R_{k+1} = inter_k + T_{k+1};
walking k = 64..1 with T_65 = 0: exact T anchors from DVE bins, F bins give
cnt_k = 2*(F_k - T_{k+1}).  ACT pipelines accumulate passes at ~1.3 us while
DVE accumulate passes have a ~4.3 us drain period, so ACT takes the relu
family plus most count bins (sigmoid) and DVE takes preprocessing plus a
spread subset of exact-count anchor bins.
"""

import os
import sys

import numpy as np

for _p in ("/opt/trn_rl_repo",):
    if os.path.isdir(_p) and _p not in sys.path:
        sys.path.insert(0, _p)

from concourse import bacc, bass, mybir, tile  # noqa: E402
from concourse import bass_utils  # noqa: E402

NUM_COMP = 64
EPS = 1e-5
B, C, H, W, D = 2, 2, 128, 128, 128
N = H * W * D
NCORES = 8
CORES_PER_SAMPLE = NCORES // B
CHUNK = N // CORES_PER_SAMPLE
P = 128
F = CHUNK // P
KMAX = NUM_COMP

# Exact-count anchor bins computed on DVE (tensor_scalar is_ge + accum).
# Spread so that sigmoid-chain reconstruction segments stay short.
_nd = int(os.environ.get("CC_ND", "58"))
if _nd >= KMAX:
    DVE_BINS = frozenset(range(1, KMAX + 1))
else:
    # evenly spread anchors from k=KMAX downward
    _step = max(1, round(KMAX / max(_nd, 1)))
    DVE_BINS = frozenset(
        k for k in range(KMAX, 0, -_step)
    ) | {KMAX}
    DVE_BINS = frozenset(sorted(DVE_BINS, reverse=True)[:max(_nd, 1)])
TRACE = False

_prog_cache = {}


def _build_program():
    nc = bacc.Bacc(
        "TRN2",
        target_bir_lowering=False,
        debug=False,
        enable_asserts=False,
        num_devices=NCORES,
    )
    f32 = mybir.dt.float32
    u8 = mybir.dt.uint8

    z0_d = nc.dram_tensor("z0", [P, F], f32, kind="ExternalInput").ap()
    z1_d = nc.dram_tensor("z1", [P, F], f32, kind="ExternalInput").ap()
    y_d = nc.dram_tensor("yb", [P, F], u8, kind="ExternalInput").ap()
    g_d = nc.dram_tensor("vor", [P, F], u8, kind="ExternalInput").ap()
    # bias constants: col j (j=0..63): -(j+1.5) for relu; col 64: 0.0
    negk_d = nc.dram_tensor("negk", [P, KMAX + 1], f32, kind="ExternalInput").ap()
    # sigmoid bias constants: col j: -30*(j+1.5)
    sigb_d = nc.dram_tensor("sigb", [P, KMAX], f32, kind="ExternalInput").ap()
    out_d = nc.dram_tensor("out", [P, 3 * KMAX], f32, kind="ExternalOutput").ap()

    Alu = mybir.AluOpType
    Act = mybir.ActivationFunctionType

    with tile.TileContext(nc) as tc:
        with tc.tile_pool(name="main", bufs=1) as pool:
            z0 = pool.tile([P, F], f32)
            z1 = pool.tile([P, F], f32)
            yt = pool.tile([P, F], u8)
            gt = pool.tile([P, F], u8)
            negk = pool.tile([P, KMAX + 1], f32)
            sigb = pool.tile([P, KMAX], f32)
            nc.sync.dma_start(out=z0[:], in_=z0_d[:])
            nc.sync.dma_start(out=z1[:], in_=z1_d[:])
            nc.sync.dma_start(out=yt[:], in_=y_d[:])
            nc.sync.dma_start(out=gt[:], in_=g_d[:])
            nc.sync.dma_start(out=negk[:], in_=negk_d[:])
            nc.sync.dma_start(out=sigb[:], in_=sigb_d[:])

            # ---- preprocessing (DVE) ----
            zt = pool.tile([P, F], f32)
            nc.vector.tensor_scalar(
                out=zt[:], in0=gt[:], scalar1=0.5, scalar2=None, op0=Alu.add,
            )
            s = pool.tile([P, F], f32)
            nc.vector.tensor_sub(s[:], z1[:], z0[:])
            yf = pool.tile([P, F], f32)
            nc.vector.tensor_scalar(
                out=yf[:], in0=yt[:], scalar1=2.0, scalar2=-1.0,
                op0=Alu.mult, op1=Alu.add,
            )
            t = pool.tile([P, F], f32)
            nc.vector.tensor_mul(t[:], s[:], yf[:])

            racc = pool.tile([P, KMAX], f32)
            tacc = pool.tile([P, KMAX], f32)
            facc = pool.tile([P, KMAX], f32)
            trash_a = pool.tile([P, F], f32)
            trash_d = pool.tile([P, F], f32)
            trash_s = pool.tile([P, F], f32)

            sig_bins = [k for k in range(1, KMAX + 1) if k not in DVE_BINS]

            # a few sigmoid count passes first: they only need zt, so ACT
            # starts ~3us in while the DVE preprocessing chain runs
            def emit_sig(k):
                j = k - 1
                nc.scalar.activation(
                    out=trash_s[:], in_=zt[:], func=Act.Sigmoid,
                    bias=sigb[:, j:j + 1], scale=30.0,
                    accum_out=facc[:, j:j + 1],
                )

            head = sig_bins[:3]
            for k in head:
                emit_sig(k)
            v = pool.tile([P, F], f32)
            nc.scalar.activation(
                out=v[:], in_=t[:], func=Act.Sigmoid,
                bias=negk[:, KMAX:KMAX + 1], scale=1.0,  # bias 0.0
            )
            for k in sig_bins[3:]:
                emit_sig(k)
            x = pool.tile([P, F], f32)
            nc.vector.tensor_add(x[:], v[:], zt[:])
            # exact count anchors on DVE
            for k in range(1, KMAX + 1):
                j = k - 1
                if k in DVE_BINS:
                    nc.vector.tensor_scalar(
                        out=trash_d[:], in0=x[:], scalar1=float(k) + 0.5,
                        scalar2=None, op0=Alu.is_ge, op1=Alu.add,
                        accum_out=tacc[:, j:j + 1],
                    )
            # relu value block on ACT (one activation-table switch total)
            for k in range(1, KMAX + 1):
                j = k - 1
                nc.scalar.activation(
                    out=trash_a[:], in_=x[:], func=Act.Relu,
                    bias=negk[:, j:j + 1], scale=1.0,
                    accum_out=racc[:, j:j + 1],
                )

            nc.sync.dma_start(out=out_d[:, 0:KMAX], in_=racc[:])
            nc.sync.dma_start(out=out_d[:, KMAX:2 * KMAX], in_=tacc[:])
            nc.sync.dma_start(out=out_d[:, 2 * KMAX:3 * KMAX], in_=facc[:])

    nc.compile()
    return nc


def _get_program():
    key = ("prog", tuple(sorted(DVE_BINS)))
    if key not in _prog_cache:
        _prog_cache[key] = _build_program()
    return _prog_cache[key]


def _consts():
    negk = np.concatenate(
        [-(np.arange(1, KMAX + 1, dtype=np.float32) + 0.5), np.zeros(1, np.float32)])
    sigb = -30.0 * (np.arange(1, KMAX + 1, dtype=np.float32) + 0.5)
    return (np.broadcast_to(negk, (P, KMAX + 1)).copy(),
            np.broadcast_to(sigb, (P, KMAX)).copy())


def kernel(y_pred: np.ndarray, y: np.ndarray, voronoi: np.ndarray) -> np.ndarray:
    y_pred = np.asarray(y_pred, dtype=np.float32)
    y = np.asarray(y)
    voronoi = np.asarray(voronoi)

    nc = _get_program()
    negk, sigb = _consts()

    in_maps = []
    for c in range(NCORES):
        b = c // CORES_PER_SAMPLE
        q = c % CORES_PER_SAMPLE
        sl = slice(q * CHUNK, (q + 1) * CHUNK)
        zp = y_pred[b].reshape(C, N)
        in_maps.append({
            "z0": np.ascontiguousarray(zp[0, sl]).reshape(P, F),
            "z1": np.ascontiguousarray(zp[1, sl]).reshape(P, F),
            "yb": np.ascontiguousarray(
                y[b, 0].reshape(N)[sl]).astype(np.uint8).reshape(P, F),
            "vor": np.ascontiguousarray(
                voronoi[b].reshape(N)[sl]).astype(np.uint8).reshape(P, F),
            "negk": negk,
            "sigb": sigb,
        })

    res = bass_utils.run_bass_kernel_spmd(
        nc, in_maps, core_ids=list(range(NCORES)), trace=TRACE,
    )
    kernel.last_results = res

    # ---- host-side gather/unshard: combine per-core partials ----
    R = np.zeros((B, KMAX + 2), dtype=np.float64)
    Tm = np.zeros((B, KMAX + 2), dtype=np.float64)
    Fm = np.zeros((B, KMAX + 2), dtype=np.float64)
    for c in range(NCORES):
        b = c // CORES_PER_SAMPLE
        out = np.asarray(res.results[c]["out"], dtype=np.float64)
        R[b, 1:KMAX + 1] += out[:, 0:KMAX].sum(axis=0)
        Tm[b, 1:KMAX + 1] += out[:, KMAX:2 * KMAX].sum(axis=0)
        Fm[b, 1:KMAX + 1] += out[:, 2 * KMAX:3 * KMAX].sum(axis=0)

    scores = []
    for b in range(B):
        cnt = np.zeros(KMAX + 2)
        T = np.zeros(KMAX + 2)          # reconstructed T_k, T_65 = 0
        for k in range(KMAX, 0, -1):
            if k in DVE_BINS:
                T[k] = Tm[b, k]
                cnt[k] = T[k] - T[k + 1]
            else:
                cnt[k] = 2.0 * (Fm[b, k] - T[k + 1])
                T[k] = T[k + 1] + cnt[k]
        k = np.arange(1, KMAX + 1)
        M = R[b, k] - R[b, k + 1]
        inter = M - T[k + 1]
        cntk = cnt[k]
        # counts are integers; snap to kill sigmoid-chain noise
        cntk = np.round(cntk)
        dice = (2.0 * inter + EPS) / (2.0 * cntk + EPS)
        present = cntk > 0
        n_present = max(present.sum(), 1)
        scores.append(np.where(present, dice, 0.0).sum() / n_present)

    return np.float32(np.mean(scores))
